# revision 1
# baseline (speedup 1.0000x reference)
"""Causal multi-head attention for TRN2, sharded across 8 NeuronCores.

Problem: x[4,2048,1024] -> 16-head causal self-attention (head_dim 64) with
QKV + output projections, fp32.

Sharding: core c -> batch b = c // 2, head-group g = c % 2 (heads g*8..g*8+7).
Per core: Q/K/V projections use the 512 weight columns of its head-group
(column-parallel); attention runs over its 8 heads; the output projection
uses the matching 512 rows of wo (row-parallel), so each core emits a
partial [2048,1024] output and the host sums the two partials per batch.
bo is added on the g==0 cores only (g==1 cores receive zeros).

Device design (per core; S=2048, D=1024, HD=64; matmul operands bf16, all
accumulation fp32 in PSUM; measured ~3e-3 relative error vs the fp32 ref):
  - x and the weights are shipped pre-transposed/pre-cast (host-side bf16)
    so the PE contracts over D with no on-device transposes or casts.
  - Q^T is computed directly as [qcol, S] (head-pair per 128-row tile).
  - K^T is stored zero-padded per head (KTz [128, 8, S]: even heads in rows
    0:64 with rows 64:128 zero, odd heads the reverse) so every score
    matmul contracts over the full 128 partitions -- K=64 matmuls measured
    2x slower on HW, and row-tiled pairing does not engage.
  - Scores are computed transposed (S^T[k,q]); exp(x/8) runs on the scalar
    engine straight out of PSUM; an all-ones column appended to each head's
    V block makes the AV matmul accumulate softmax denominators in psum row
    64. V blocks are padded to 128 lhsT columns (zeros) so the AV matmul is
    a full 128x128 stationary shape. Causal structure: each k-tile only
    covers its valid q-range; only the 128x128 diagonal block is masked
    (DVE multiply by an upper-triangular tile).
  - Softmax normalization per head: DVE copy of the denominator row to
    SBUF, reciprocal_approx_fast (reading PSUM directly is broken on HW),
    gpsimd partition-broadcast, one DVE multiply into A^T.
  - Biases: bq/bk fold into the PSUM->SBUF copies as per-partition
    tensor_scalar adds; bv/bo are partition-broadcast once and folded into
    the V/out copies as tensor_tensor adds (no rank-1 bias matmuls).
  - Single-scope software pipeline: chunk j+1's projection psum-groups and
    chunk j-1's output-projection groups are interleaved between the heads
    of attention chunk j as PE filler, so the exp-bound stretches stay
    covered; only chunk 3's output projection runs as tail.
"""

import os
from contextlib import ExitStack

import numpy as np

import concourse.bacc as bacc
import concourse.mybir as mybir
import concourse.tile as tile
from concourse.bass_utils import run_bass_kernel_spmd
from concourse.masks import make_upper_triangular

F32 = mybir.dt.float32
F32R = mybir.dt.float32r
BF16 = mybir.dt.bfloat16
AF = mybir.ActivationFunctionType
ALU = mybir.AluOpType

B = 4
S = 2048
D = 1024
HD = 64
HG = 8  # heads per core
QC = HG * HD  # 512 local q/k/v columns
N_CORES = 8

_NC_CACHE = {}
LAST_RESULT = None  # BassKernelResults of the most recent kernel() call


def _build_nc(s: int = S, num_devices: int = N_CORES):
    P = 128
    NQ = s // 512
    NS = s // P
    ND = D // P
    NT = QC // P
    VW = HD + 1  # 65: per-head V block width (64 cols + ones col)
    VPAD = 7 * VW + P  # 583: last head's lhsT slice must fit

    nc = bacc.Bacc("TRN2", target_bir_lowering=False, debug=False, num_devices=num_devices)

    xT_d = nc.dram_tensor("xT", [D, s], BF16, kind="ExternalInput").ap()
    wq_d = nc.dram_tensor("wq", [D, QC], BF16, kind="ExternalInput").ap()
    wk_d = nc.dram_tensor("wk", [D, QC], BF16, kind="ExternalInput").ap()
    wv_d = nc.dram_tensor("wv", [D, QC], BF16, kind="ExternalInput").ap()
    wo_d = nc.dram_tensor("wo", [QC, D], BF16, kind="ExternalInput").ap()
    bq_d = nc.dram_tensor("bq", [QC], F32, kind="ExternalInput").ap()
    bk_d = nc.dram_tensor("bk", [QC], F32, kind="ExternalInput").ap()
    bv_d = nc.dram_tensor("bv", [QC], F32, kind="ExternalInput").ap()
    bo_d = nc.dram_tensor("bo", [D], F32, kind="ExternalInput").ap()
    ones_d = nc.dram_tensor("ones2d", [P, 512], F32, kind="ExternalInput").ap()
    zeros_d = nc.dram_tensor("zeros2d", [P, 4096], F32, kind="ExternalInput").ap()
    out_d = nc.dram_tensor("out", [s, D], F32, kind="ExternalOutput").ap()

    with tile.TileContext(nc) as tc:
        with ExitStack() as ctx:
            consts = ctx.enter_context(tc.tile_pool(name="consts", bufs=1))
            persist = ctx.enter_context(tc.tile_pool(name="persist", bufs=1))
            e_pool = ctx.enter_context(tc.tile_pool(name="epool", bufs=8))
            n_pool = ctx.enter_context(tc.tile_pool(name="npool", bufs=4))
            b_pool = ctx.enter_context(tc.tile_pool(name="bpool", bufs=4))
            o_pool = ctx.enter_context(tc.tile_pool(name="opool", bufs=3))
            proj_psum = ctx.enter_context(tc.tile_pool(name="proj_ps", bufs=2, space="PSUM"))
            s_psum = ctx.enter_context(tc.tile_pool(name="s_ps", bufs=3, space="PSUM"))
            a_psum = ctx.enter_context(tc.tile_pool(name="a_ps", bufs=2, space="PSUM"))
            o_psum = ctx.enter_context(tc.tile_pool(name="o_ps", bufs=1, space="PSUM"))

            ones_t = consts.tile([P, 512], F32)
            nc.sync.dma_start(ones_t[:], ones_d)
            tri = consts.tile([P, P], F32)
            make_upper_triangular(nc, tri[:], val=1.0, diag=True)
            tri_b = consts.tile([P, P], BF16)
            nc.any.tensor_copy(tri_b[:], tri[:])
            bqc = consts.tile([P, NT], F32)
            bkc = consts.tile([P, NT], F32)
            nc.sync.dma_start(bqc[:], bq_d.rearrange("(t p) -> p t", p=P))
            nc.sync.dma_start(bkc[:], bk_d.rearrange("(t p) -> p t", p=P))
            bv1 = consts.tile([1, QC], F32)
            bo1 = consts.tile([1, D], F32)
            nc.sync.dma_start(bv1[:], bv_d[None, :])
            nc.sync.dma_start(bo1[:], bo_d[None, :])
            bvb = consts.tile([P, QC], F32)
            bob = consts.tile([P, D], F32)
            nc.gpsimd.partition_broadcast(bvb[:], bv1[0:1, :])
            nc.gpsimd.partition_broadcast(bob[:], bo1[0:1, :])

            QT = persist.tile([P, NT, s], BF16)
            KTz = persist.tile([P, HG, s], BF16)
            V = persist.tile([P, NS, VPAD + 1], BF16)
            AT = persist.tile([P, NT, s], BF16)
            xT = persist.tile([P, ND, s], BF16)
            wq_sb = persist.tile([P, ND, QC], BF16)
            wk_sb = persist.tile([P, ND, QC], BF16)
            wv_sb = persist.tile([P, ND, QC], BF16)
            wo_sb = persist.tile([P, NT, D], BF16)

            nc.sync.dma_start(wq_sb[:], wq_d.rearrange("(nd p) c -> p nd c", p=P))
            for d in range(ND):
                nc.sync.dma_start(xT[:, d, :], xT_d[d * P : (d + 1) * P, :])
            nc.sync.dma_start(wk_sb[:], wk_d.rearrange("(nd p) c -> p nd c", p=P))
            nc.sync.dma_start(wv_sb[:], wv_d.rearrange("(nd p) c -> p nd c", p=P))
            nc.sync.dma_start(wo_sb[:], wo_d.rearrange("(nt p) c -> p nt c", p=P))

            # zero the pad rows of KTz; zero V pad cols; set V ones columns
            zee = zeros_d.bitcast(BF16)
            for t in range(NT):
                nc.sync.dma_start(KTz[64:128, 2 * t, :], zee[64:128, 0:s])
                nc.sync.dma_start(KTz[0:64, 2 * t + 1, :], zee[0:64, 0:s])
            nc.sync.dma_start(
                V[:, :, 7 * VW + HD + 1 :],
                zee[:, 0 : NS * (P - HD)].rearrange("p (a b) -> p a b", a=NS),
            )
            nc.any.tensor_copy(
                V[:, :, 0 : HG * VW].rearrange("p s (h c) -> p s h c", c=VW)[:, :, :, HD : HD + 1],
                ones_t[:, 0 : NS * HG].rearrange("p (a b c) -> p a b c", a=NS, b=HG),
            )

            warm = o_psum.tile([P, 512], F32, tag="o_ps", name="warm")
            for _ in range(32):
                nc.tensor.matmul(
                    warm[:, 0:P], lhsT=tri_b[:], rhs=tri_b[:], start=True, stop=True
                )

            def proj_group(j, g):
                """One psum-group of the j-chunk projections; g in 0..11."""
                js = slice(j * 512, (j + 1) * 512)
                kind, t = divmod(g, NT)
                ps = proj_psum.tile([P, 512], F32, tag="pp", name="pp")
                if kind == 0:  # Q
                    for d in range(ND):
                        nc.tensor.matmul(
                            ps[:],
                            lhsT=wq_sb[:, d, t * P : (t + 1) * P],
                            rhs=xT[:, d, js],
                            start=(d == 0),
                            stop=(d == ND - 1),
                        )
                    nc.vector.tensor_scalar_add(QT[:, t, js], ps[:], bqc[:, t : t + 1])
                elif kind == 1:  # K
                    for d in range(ND):
                        nc.tensor.matmul(
                            ps[:],
                            lhsT=wk_sb[:, d, t * P : (t + 1) * P],
                            rhs=xT[:, d, js],
                            start=(d == 0),
                            stop=(d == ND - 1),
                        )
                    nc.vector.tensor_scalar_add(
                        KTz[0:64, 2 * t, js], ps[0:64, :], bkc[0:64, t : t + 1]
                    )
                    nc.vector.tensor_scalar_add(
                        KTz[64:128, 2 * t + 1, js], ps[64:128, :], bkc[64:128, t : t + 1]
                    )
                else:  # V s-tile 4j+t
                    st = 4 * j + t
                    for d in range(ND):
                        nc.tensor.matmul(
                            ps[:],
                            lhsT=xT[:, d, st * P : (st + 1) * P],
                            rhs=wv_sb[:, d, :],
                            start=(d == 0),
                            stop=(d == ND - 1),
                        )
                    dst = V[:, st, 0 : HG * VW].rearrange("p (h c) -> p h c", c=VW)[:, :, 0:HD]
                    src = ps.rearrange("p (h c) -> p h c", c=HD)
                    bsrc = bvb.rearrange("p (h c) -> p h c", c=HD)
                    nc.vector.tensor_tensor(dst, src, bsrc, ALU.add)

            def attn_head(j, h):
                t, half = h // 2, h % 2
                pb = 64 * half
                nkb = 4 * j + 4
                A_ps = a_psum.tile([P, 512], F32, tag="A", name="A")
                for kb in range(nkb):
                    y0 = max(0, P * (kb - 4 * j))
                    s_ps = s_psum.tile([P, 512], F32, name="s_ps")
                    nc.tensor.matmul(
                        s_ps[:, y0:],
                        lhsT=KTz[:, h, kb * P : (kb + 1) * P],
                        rhs=QT[:, t, j * 512 + y0 : (j + 1) * 512],
                        start=True,
                        stop=True,
                    )
                    E = e_pool.tile([P, 512], BF16, name="E")
                    nc.scalar.activation(E[:, y0:], s_ps[:, y0:], AF.Exp, scale=0.125)
                    if kb >= 4 * j:
                        nc.vector.tensor_tensor(
                            E[:, y0 : y0 + P], E[:, y0 : y0 + P], tri_b[:], ALU.mult
                        )
                    nc.tensor.matmul(
                        A_ps[:, y0:],
                        lhsT=V[:, kb, h * VW : h * VW + P],
                        rhs=E[:, y0:],
                        start=(kb == 0),
                        stop=(kb == nkb - 1),
                    )
                sums = n_pool.tile([1, 512], F32, tag="sums", name="sums")
                nc.vector.tensor_copy(sums[:], A_ps[HD : HD + 1, :])
                rec = n_pool.tile([1, 512], F32, tag="rec", name="rec")
                nc.vector.reciprocal_approx_fast(rec[:], sums[:])
                bc = b_pool.tile([HD, 512], F32, name="bc")
                nc.gpsimd.partition_broadcast(bc[:], rec[0:1, :])
                nc.vector.tensor_tensor(
                    AT[pb : pb + HD, t, j * 512 : (j + 1) * 512],
                    A_ps[0:HD, :],
                    bc[:],
                    ALU.mult,
                )

            def out_proj_group(j, g, pool=None):
                st = 4 * j + g // 2
                oc = g % 2
                o_ps = (pool or o_psum).tile(
                    [P, 512], F32, tag="pp" if pool is not None else "o_ps", name="o_ps"
                )
                for t2 in range(NT):
                    nc.tensor.matmul(
                        o_ps[:],
                        lhsT=AT[:, t2, st * P : (st + 1) * P],
                        rhs=wo_sb[:, t2, oc * 512 : (oc + 1) * 512],
                        start=(t2 == 0),
                        stop=(t2 == NT - 1),
                    )
                ot = o_pool.tile([P, 512], F32, name="ot")
                nc.vector.tensor_tensor(
                    ot[:], o_ps[:], bob[:, oc * 512 : (oc + 1) * 512], ALU.add
                )
                nc.sync.dma_start(
                    out_d[st * P : (st + 1) * P, oc * 512 : (oc + 1) * 512], ot[:]
                )

            # j-chunk 0 projections up front; then pipeline: attention(j)
            # interleaved with the projections of chunk j+1 at head granularity
            for g in range(12):
                proj_group(0, g)
            for j in range(NQ):
                # filler between heads: chunk j+1 projections, then chunk j-1
                # output projection (ready since attn(j-1) completed)
                filler = [("p", j + 1, g) for g in range(12)] if j + 1 < NQ else []
                if j > 0:
                    filler += [("o", j - 1, g) for g in range(8)]
                for h in range(HG):
                    attn_head(j, h)
                    k0 = (len(filler) * h) // HG
                    k1 = (len(filler) * (h + 1)) // HG
                    for kind, jj, g in filler[k0:k1]:
                        if kind == "p":
                            proj_group(jj, g)
                        else:
                            out_proj_group(jj, g)
                    if j == NQ - 1 and h < HG - 1:
                        # no filler work in the last chunk: two dummy matmuls
                        # keep the PE HAM clock gate warm across exp waits
                        kw = o_psum.tile([P, 512], F32, tag="o_ps", name="kw")
                        for _ in range(2):
                            nc.tensor.matmul(
                                kw[:, 0:P], lhsT=tri_b[:], rhs=tri_b[:],
                                start=True, stop=True,
                            )
            # tail: projections are done, so alternate the out-proj groups
            # into the now-idle proj_psum to double-buffer the tail
            for g in range(8):
                out_proj_group(NQ - 1, g, pool=proj_psum if g % 2 else None)

    nc.compile()

    return nc


def _get_nc():
    if "nc" not in _NC_CACHE:
        _NC_CACHE["nc"] = _build_nc()
    return _NC_CACHE["nc"]


def make_in_maps(x, wq, bq, wk, bk, wv, bv, wo, bo, n_cores=N_CORES):
    import ml_dtypes

    bf = ml_dtypes.bfloat16
    x = np.asarray(x, np.float32).astype(bf)
    wq, wk, wv, wo = (np.asarray(a, np.float32).astype(bf) for a in (wq, wk, wv, wo))
    bq, bk, bv, bo = (np.asarray(a, np.float32) for a in (bq, bk, bv, bo))
    ones2d = np.ones((128, 512), np.float32)
    zeros2d = np.zeros((128, 4096), np.float32)
    in_maps = []
    for c in range(n_cores):
        b, g = c // 2, c % 2
        cs = slice(g * QC, (g + 1) * QC)
        in_maps.append(
            {
                "xT": np.ascontiguousarray(x[b].T),
                "wq": np.ascontiguousarray(wq[:, cs]),
                "wk": np.ascontiguousarray(wk[:, cs]),
                "wv": np.ascontiguousarray(wv[:, cs]),
                "wo": np.ascontiguousarray(wo[cs, :]),
                "bq": np.ascontiguousarray(bq[cs]),
                "bk": np.ascontiguousarray(bk[cs]),
                "bv": np.ascontiguousarray(bv[cs]),
                "bo": bo if g == 0 else np.zeros_like(bo),
                "ones2d": ones2d,
                "zeros2d": zeros2d,
            }
        )
    return in_maps


def kernel(x, wq, bq, wk, bk, wv, bv, wo, bo):
    global LAST_RESULT
    in_maps = make_in_maps(x, wq, bq, wk, bk, wv, bv, wo, bo)
    nc = _get_nc()
    trace = os.environ.get("MHA_TRACE", "0") == "1"
    res = run_bass_kernel_spmd(nc, in_maps, core_ids=list(range(N_CORES)), trace=trace)
    LAST_RESULT = res

    out = np.empty((B, S, D), np.float32)
    for b in range(B):
        out[b] = res.results[2 * b]["out"] + res.results[2 * b + 1]["out"]
    return out



# revision 2
# speedup vs baseline: 1.1732x; 1.1732x over previous
"""Causal multi-head attention for TRN2, sharded across 8 NeuronCores.

Problem: x[4,2048,1024] -> 16-head causal self-attention (head_dim 64) with
QKV + output projections, fp32.

Sharding: core c -> batch b = c // 2, head-group g = c % 2 (heads g*8..g*8+7).
Per core: Q/K/V projections use the 512 weight columns of its head-group
(column-parallel); attention runs over its 8 heads; the output projection
uses the matching 512 rows of wo (row-parallel), so each core emits a
partial [2048,1024] output and the host sums the two partials per batch.

Math simplifications vs the reference (both exact):
  - bk is dropped: softmax over k is invariant to the per-q shift
    (q+bq)@bk that the K bias induces.
  - bv is folded into the output bias host-side (bo_eff = bo + bv@wo,
    per-core bv[cs]@wo[cs,:]); the V bias passes through softmax
    normalization unchanged because attention weights sum to 1.

Device design (per core; S=2048, D=1024, HD=64; matmul operands bf16, all
accumulation fp32 in PSUM):
  - x and the weights are shipped pre-transposed/pre-cast (host-side bf16)
    so the PE contracts over D with no on-device transposes or casts.
  - Q^T is computed directly as [qcol, S] (head-pair per 128-row tile).
  - K^T is stored zero-padded per head (KTz [128, 8, S]: even heads in rows
    0:64 with rows 64:128 zero, odd heads the reverse) so every score
    matmul contracts over the full 128 partitions -- K=64 matmuls measured
    2x slower on HW (no row pairing).
  - Scores are computed transposed (S^T[k,q]); exp(x/8) runs on the scalar
    engine straight out of PSUM. Score PSUM tiles are [128,2,512] PAIRS
    spanning two banks: two k-blocks' scores get ONE exp instruction over
    1024 columns, halving the ~400ns/instr activation overhead that
    dominated the scalar engine (245us -> ~180us busy).
  - An all-ones column appended to each head's V block makes the AV matmul
    accumulate softmax denominators in psum row 64. V blocks are padded to
    128 lhsT columns. Causal: each k-tile covers only its valid q-range;
    only the 128x128 diagonal block is masked (DVE multiply by tri).
  - Softmax normalization per head: DVE copy of the denominator row to
    SBUF, reciprocal_approx_fast (reading PSUM directly is broken on HW),
    gpsimd partition-broadcast, one DVE multiply into A^T.
  - PSUM budget: scores 2 pair-tiles (4 banks) + A 2 + proj/out shared 2.
  - Single-scope software pipeline: chunk j+1's projection psum-groups and
    chunk j-1's output-projection groups are interleaved between the heads
    of attention chunk j as PE filler; only chunk 3's output projection
    runs as tail.
  - Startup: PE warmup matmuls depend only on the small ones DMA; x and wq
    stream per-128-row tile so the first projection matmuls start early.
"""

import os
from contextlib import ExitStack

import numpy as np

import concourse.bacc as bacc
import concourse.mybir as mybir
import concourse.tile as tile
from concourse.bass_utils import run_bass_kernel_spmd
from concourse.masks import make_upper_triangular

F32 = mybir.dt.float32
BF16 = mybir.dt.bfloat16
AF = mybir.ActivationFunctionType
ALU = mybir.AluOpType

B = 4
S = 2048
D = 1024
HD = 64
HG = 8  # heads per core
QC = HG * HD  # 512 local q/k/v columns
N_CORES = 8

_NC_CACHE = {}
LAST_RESULT = None  # BassKernelResults of the most recent kernel() call


def _build_nc(s: int = S, num_devices: int = N_CORES):
    P = 128
    NQ = s // 512
    NS = s // P
    ND = D // P
    NT = QC // P
    VW = HD + 1  # 65: per-head V block width (64 cols + ones col)
    VPAD = 7 * VW + P  # 583: last head's lhsT slice must fit

    nc = bacc.Bacc("TRN2", target_bir_lowering=False, debug=False, num_devices=num_devices)

    xT_d = nc.dram_tensor("xT", [D, s], BF16, kind="ExternalInput").ap()
    wq_d = nc.dram_tensor("wq", [D, QC], BF16, kind="ExternalInput").ap()
    wk_d = nc.dram_tensor("wk", [D, QC], BF16, kind="ExternalInput").ap()
    wv_d = nc.dram_tensor("wv", [D, QC], BF16, kind="ExternalInput").ap()
    wo_d = nc.dram_tensor("wo", [QC, D], BF16, kind="ExternalInput").ap()
    bq_d = nc.dram_tensor("bq", [QC], F32, kind="ExternalInput").ap()
    bo_d = nc.dram_tensor("bo", [D], F32, kind="ExternalInput").ap()
    ones_d = nc.dram_tensor("ones2d", [P, 512], F32, kind="ExternalInput").ap()
    zeros_d = nc.dram_tensor("zeros2d", [P, 4096], F32, kind="ExternalInput").ap()
    out_d = nc.dram_tensor("out", [s, D], F32, kind="ExternalOutput").ap()

    with tile.TileContext(nc) as tc:
        with ExitStack() as ctx:
            consts = ctx.enter_context(tc.tile_pool(name="consts", bufs=1))
            persist = ctx.enter_context(tc.tile_pool(name="persist", bufs=1))
            e_pool = ctx.enter_context(tc.tile_pool(name="epool", bufs=5))
            n_pool = ctx.enter_context(tc.tile_pool(name="npool", bufs=4))
            b_pool = ctx.enter_context(tc.tile_pool(name="bpool", bufs=4))
            o_pool = ctx.enter_context(tc.tile_pool(name="opool", bufs=3))
            proj_psum = ctx.enter_context(tc.tile_pool(name="proj_ps", bufs=2, space="PSUM"))
            s_psum = ctx.enter_context(tc.tile_pool(name="s_ps", bufs=2, space="PSUM"))
            a_psum = ctx.enter_context(tc.tile_pool(name="a_ps", bufs=2, space="PSUM"))

            # --- warmup path: depends only on the small ones DMA ---
            ones_t = consts.tile([P, 512], F32)
            nc.sync.dma_start(ones_t[:], ones_d)
            ones_b = consts.tile([P, P], BF16)
            nc.any.tensor_copy(ones_b[:], ones_t[:, 0:P])

            warm = proj_psum.tile([P, 512], F32, tag="pp", name="warm")
            for _ in range(32):
                nc.tensor.matmul(
                    warm[:, 0:P], lhsT=ones_b[:], rhs=ones_b[:], start=True, stop=True
                )

            # --- input DMAs, ordered by first use ---
            bqc = consts.tile([P, NT], F32)
            nc.sync.dma_start(bqc[:], bq_d.rearrange("(t p) -> p t", p=P))

            QT = persist.tile([P, NT, s], BF16)
            KTz = persist.tile([P, HG, s], BF16)
            V = persist.tile([P, NS, VPAD + 1], BF16)
            AT = persist.tile([P, NT, s], BF16)
            xT = persist.tile([P, ND, s], BF16)
            wq_sb = persist.tile([P, ND, QC], BF16)
            wk_sb = persist.tile([P, ND, QC], BF16)
            wv_sb = persist.tile([P, ND, QC], BF16)
            wo_sb = persist.tile([P, NT, D], BF16)

            for d in range(ND):
                nc.sync.dma_start(wq_sb[:, d, :], wq_d[d * P : (d + 1) * P, :])
                nc.sync.dma_start(xT[:, d, :], xT_d[d * P : (d + 1) * P, :])
            for d in range(ND):
                nc.sync.dma_start(wk_sb[:, d, :], wk_d[d * P : (d + 1) * P, :])
            for d in range(ND):
                nc.sync.dma_start(wv_sb[:, d, :], wv_d[d * P : (d + 1) * P, :])

            # zero the pad rows of KTz; zero V pad cols; set V ones columns
            zee = zeros_d.bitcast(BF16)
            for t in range(NT):
                nc.sync.dma_start(KTz[64:128, 2 * t, :], zee[64:128, 0:s])
                nc.sync.dma_start(KTz[0:64, 2 * t + 1, :], zee[0:64, 0:s])
            nc.sync.dma_start(
                V[:, :, 7 * VW + HD + 1 :],
                zee[:, 0 : NS * (P - HD)].rearrange("p (a b) -> p a b", a=NS),
            )
            nc.any.tensor_copy(
                V[:, :, 0 : HG * VW].rearrange("p s (h c) -> p s h c", c=VW)[:, :, :, HD : HD + 1],
                ones_t[:, 0 : NS * HG].rearrange("p (a b c) -> p a b c", a=NS, b=HG),
            )

            nc.sync.dma_start(wo_sb[:], wo_d.rearrange("(nt p) c -> p nt c", p=P))
            bo1 = consts.tile([1, D], F32)
            nc.sync.dma_start(bo1[:], bo_d[None, :])
            bob = consts.tile([P, D], F32)
            nc.gpsimd.partition_broadcast(bob[:], bo1[0:1, :])

            tri = consts.tile([P, P], F32)
            make_upper_triangular(nc, tri[:], val=1.0, diag=True)
            tri_b = consts.tile([P, P], BF16)
            nc.any.tensor_copy(tri_b[:], tri[:])

            def proj_group(j, g):
                """One psum-group of the j-chunk projections; g in 0..11."""
                js = slice(j * 512, (j + 1) * 512)
                kind, t = divmod(g, NT)
                ps = proj_psum.tile([P, 512], F32, tag="pp", name="pp")
                if kind == 0:  # Q
                    for d in range(ND):
                        nc.tensor.matmul(
                            ps[:],
                            lhsT=wq_sb[:, d, t * P : (t + 1) * P],
                            rhs=xT[:, d, js],
                            start=(d == 0),
                            stop=(d == ND - 1),
                        )
                    nc.vector.tensor_scalar_add(QT[:, t, js], ps[:], bqc[:, t : t + 1])
                elif kind == 1:  # K (no bias: softmax shift-invariance)
                    for d in range(ND):
                        nc.tensor.matmul(
                            ps[:],
                            lhsT=wk_sb[:, d, t * P : (t + 1) * P],
                            rhs=xT[:, d, js],
                            start=(d == 0),
                            stop=(d == ND - 1),
                        )
                    nc.vector.tensor_copy(KTz[0:64, 2 * t, js], ps[0:64, :])
                    nc.vector.tensor_copy(KTz[64:128, 2 * t + 1, js], ps[64:128, :])
                else:  # V s-tile 4j+t (no bias: folded into bo_eff host-side)
                    st = 4 * j + t
                    for d in range(ND):
                        nc.tensor.matmul(
                            ps[:],
                            lhsT=xT[:, d, st * P : (st + 1) * P],
                            rhs=wv_sb[:, d, :],
                            start=(d == 0),
                            stop=(d == ND - 1),
                        )
                    dst = V[:, st, 0 : HG * VW].rearrange("p (h c) -> p h c", c=VW)[:, :, 0:HD]
                    src = ps.rearrange("p (h c) -> p h c", c=HD)
                    nc.vector.tensor_copy(dst, src)

            def attn_head(j, h):
                t, half = h // 2, h % 2
                pb = 64 * half
                nkb = 4 * j + 4
                nfull = 4 * j
                js0 = j * 512
                A_ps = a_psum.tile([P, 512], F32, tag="A", name="A")
                # full k-blocks, in pairs sharing one 2-bank psum tile + one exp
                for p0 in range(0, nfull, 2):
                    s2 = s_psum.tile([P, 2, 512], F32, name="s2")
                    for i in (0, 1):
                        kb = p0 + i
                        nc.tensor.matmul(
                            s2[:, i, :],
                            lhsT=KTz[:, h, kb * P : (kb + 1) * P],
                            rhs=QT[:, t, js0 : js0 + 512],
                            start=True,
                            stop=True,
                        )
                    E = e_pool.tile([P, 2, 512], BF16, name="E")
                    nc.scalar.activation(E[:], s2[:], AF.Exp, scale=0.125)
                    for i in (0, 1):
                        kb = p0 + i
                        nc.tensor.matmul(
                            A_ps[:],
                            lhsT=V[:, kb, h * VW : h * VW + P],
                            rhs=E[:, i, :],
                            start=(kb == 0),
                            stop=False,
                        )
                # diagonal k-blocks: pair psum tile, separate trimmed exps
                for p0 in range(nfull, nkb, 2):
                    s2 = s_psum.tile([P, 2, 512], F32, name="s2")
                    for i in (0, 1):
                        kb = p0 + i
                        y0 = P * (kb - 4 * j)
                        nc.tensor.matmul(
                            s2[:, i, y0:],
                            lhsT=KTz[:, h, kb * P : (kb + 1) * P],
                            rhs=QT[:, t, js0 + y0 : js0 + 512],
                            start=True,
                            stop=True,
                        )
                    E = e_pool.tile([P, 2, 512], BF16, name="E")
                    for i in (0, 1):
                        kb = p0 + i
                        y0 = P * (kb - 4 * j)
                        nc.scalar.activation(E[:, i, y0:], s2[:, i, y0:], AF.Exp, scale=0.125)
                        nc.vector.tensor_tensor(
                            E[:, i, y0 : y0 + P], E[:, i, y0 : y0 + P], tri_b[:], ALU.mult
                        )
                        nc.tensor.matmul(
                            A_ps[:, y0:],
                            lhsT=V[:, kb, h * VW : h * VW + P],
                            rhs=E[:, i, y0:],
                            start=(kb == 0),
                            stop=(kb == nkb - 1),
                        )
                sums = n_pool.tile([1, 512], F32, tag="sums", name="sums")
                nc.vector.tensor_copy(sums[:], A_ps[HD : HD + 1, :])
                rec = n_pool.tile([1, 512], F32, tag="rec", name="rec")
                nc.vector.reciprocal_approx_fast(rec[:], sums[:])
                bc = b_pool.tile([HD, 512], F32, name="bc")
                nc.gpsimd.partition_broadcast(bc[:], rec[0:1, :])
                nc.vector.tensor_tensor(
                    AT[pb : pb + HD, t, js0 : js0 + 512],
                    A_ps[0:HD, :],
                    bc[:],
                    ALU.mult,
                )

            def out_proj_group(j, g):
                st = 4 * j + g // 2
                oc = g % 2
                o_ps = proj_psum.tile([P, 512], F32, tag="pp", name="o_ps")
                for t2 in range(NT):
                    nc.tensor.matmul(
                        o_ps[:],
                        lhsT=AT[:, t2, st * P : (st + 1) * P],
                        rhs=wo_sb[:, t2, oc * 512 : (oc + 1) * 512],
                        start=(t2 == 0),
                        stop=(t2 == NT - 1),
                    )
                ot = o_pool.tile([P, 512], F32, name="ot")
                nc.vector.tensor_tensor(
                    ot[:], o_ps[:], bob[:, oc * 512 : (oc + 1) * 512], ALU.add
                )
                nc.sync.dma_start(
                    out_d[st * P : (st + 1) * P, oc * 512 : (oc + 1) * 512], ot[:]
                )

            # j-chunk 0 projections up front; then pipeline: attention(j)
            # interleaved with the projections of chunk j+1 at head granularity
            for g in range(12):
                proj_group(0, g)
            for j in range(NQ):
                # filler between heads: chunk j+1 projections, then chunk j-1
                # output projection (ready since attn(j-1) completed)
                filler = [("p", j + 1, g) for g in range(12)] if j + 1 < NQ else []
                if j > 0:
                    filler += [("o", j - 1, g) for g in range(8)]
                for h in range(HG):
                    attn_head(j, h)
                    k0 = (len(filler) * h) // HG
                    k1 = (len(filler) * (h + 1)) // HG
                    for kind, jj, g in filler[k0:k1]:
                        if kind == "p":
                            proj_group(jj, g)
                        else:
                            out_proj_group(jj, g)
                    if j == NQ - 1 and h < HG - 1:
                        # no filler work in the last chunk: two dummy matmuls
                        # keep the PE HAM clock gate warm across exp waits
                        kw = proj_psum.tile([P, 512], F32, tag="pp", name="kw")
                        for _ in range(2):
                            nc.tensor.matmul(
                                kw[:, 0:P], lhsT=ones_b[:], rhs=ones_b[:],
                                start=True, stop=True,
                            )
            # tail: chunk 3's output projection, double-buffered in proj_psum
            for g in range(8):
                out_proj_group(NQ - 1, g)

    nc.compile()

    return nc


def _get_nc():
    if "nc" not in _NC_CACHE:
        _NC_CACHE["nc"] = _build_nc()
    return _NC_CACHE["nc"]


def make_in_maps(x, wq, bq, wk, bk, wv, bv, wo, bo, n_cores=N_CORES):
    import ml_dtypes

    bf = ml_dtypes.bfloat16
    x = np.asarray(x, np.float32).astype(bf)
    wo_f = np.asarray(wo, np.float32)
    bv_f = np.asarray(bv, np.float32)
    wq, wk, wv, wo = (np.asarray(a, np.float32).astype(bf) for a in (wq, wk, wv, wo))
    bq, bo = (np.asarray(a, np.float32) for a in (bq, bo))
    ones2d = np.ones((128, 512), np.float32)
    zeros2d = np.zeros((128, 4096), np.float32)
    in_maps = []
    for c in range(n_cores):
        b, g = c // 2, c % 2
        cs = slice(g * QC, (g + 1) * QC)
        # bv folded into the output bias: bo_eff = bv[cs] @ wo[cs,:] (+bo on g==0)
        bo_eff = bv_f[cs] @ wo_f[cs, :]
        if g == 0:
            bo_eff = bo_eff + bo
        in_maps.append(
            {
                "xT": np.ascontiguousarray(x[b].T),
                "wq": np.ascontiguousarray(wq[:, cs]),
                "wk": np.ascontiguousarray(wk[:, cs]),
                "wv": np.ascontiguousarray(wv[:, cs]),
                "wo": np.ascontiguousarray(wo[cs, :]),
                "bq": np.ascontiguousarray(bq[cs]),
                "bo": np.ascontiguousarray(bo_eff.astype(np.float32)),
                "ones2d": ones2d,
                "zeros2d": zeros2d,
            }
        )
    return in_maps


def kernel(x, wq, bq, wk, bk, wv, bv, wo, bo):
    global LAST_RESULT
    in_maps = make_in_maps(x, wq, bq, wk, bk, wv, bv, wo, bo)
    nc = _get_nc()
    trace = os.environ.get("MHA_TRACE", "0") == "1"
    res = run_bass_kernel_spmd(nc, in_maps, core_ids=list(range(N_CORES)), trace=trace)
    LAST_RESULT = res

    out = np.empty((B, S, D), np.float32)
    for b in range(B):
        out[b] = res.results[2 * b]["out"] + res.results[2 * b + 1]["out"]
    return out


# revision 11
# speedup vs baseline: 1.1867x; 1.0115x over previous
"""Causal multi-head attention for TRN2, sharded across 8 NeuronCores.

Problem: x[4,2048,1024] -> 16-head causal self-attention (head_dim 64) with
QKV + output projections, fp32.

Sharding: core c -> batch b = c // 2, head-group g = c % 2 (heads g*8..g*8+7).
Per core: Q/K/V projections use the 512 weight columns of its head-group
(column-parallel); attention runs over its 8 heads; the output projection
uses the matching 512 rows of wo (row-parallel), so each core emits a
partial [2048,1024] output and the host sums the two partials per batch.

Math simplifications vs the reference (both exact):
  - bk is dropped: softmax over k is invariant to the per-q shift
    (q+bq)@bk that the K bias induces.
  - bv is folded into the output bias host-side (bo_eff = bo + bv@wo,
    per-core bv[cs]@wo[cs,:]); the V bias passes through softmax
    normalization unchanged because attention weights sum to 1.

Device design (per core; S=2048, D=1024, HD=64; matmul operands bf16, all
accumulation fp32 in PSUM):
  - x and the weights are shipped pre-transposed/pre-cast (host-side bf16)
    so the PE contracts over D with no on-device transposes or casts.
  - Q^T is computed directly as [qcol, S] (head-pair per 128-row tile).
  - K^T is stored zero-padded per head (KTz [128, 8, S]: even heads in rows
    0:64 with rows 64:128 zero, odd heads the reverse) so every score
    matmul contracts over the full 128 partitions -- K=64 matmuls measured
    2x slower on HW (no row pairing).
  - Scores are computed transposed (S^T[k,q]); exp(x/8) runs on the scalar
    engine straight out of PSUM. Score PSUM tiles are [128,2,512] PAIRS
    spanning two banks: two k-blocks' scores get ONE exp instruction over
    1024 columns, halving the ~400ns/instr activation overhead that
    dominated the scalar engine (245us -> ~180us busy).
  - An all-ones column appended to each head's V block makes the AV matmul
    accumulate softmax denominators in psum row 64. V blocks are padded to
    128 lhsT columns. Causal: each k-tile covers only its valid q-range;
    only the 128x128 diagonal block is masked (DVE multiply by tri).
  - Softmax normalization per head: DVE copy of the denominator row to
    SBUF, reciprocal_approx_fast (reading PSUM directly is broken on HW),
    gpsimd partition-broadcast, one DVE multiply into A^T.
  - PSUM budget: scores 2 pair-tiles (4 banks) + A 2 + proj/out shared 2.
  - Single-scope software pipeline: chunk j+1's projection psum-groups and
    chunk j-1's output-projection groups are interleaved between the heads
    of attention chunk j as PE filler; only chunk 3's output projection
    runs as tail.
  - Startup: PE warmup matmuls depend only on the small ones DMA; x and wq
    stream per-128-row tile so the first projection matmuls start early.
"""

import os
from contextlib import ExitStack

import numpy as np

import concourse.bacc as bacc
import concourse.mybir as mybir
import concourse.tile as tile
from concourse.bass_utils import run_bass_kernel_spmd
from concourse.masks import make_upper_triangular

F32 = mybir.dt.float32
BF16 = mybir.dt.bfloat16
AF = mybir.ActivationFunctionType
ALU = mybir.AluOpType

B = 4
S = 2048
D = 1024
HD = 64
HG = 8  # heads per core
QC = HG * HD  # 512 local q/k/v columns
N_CORES = 8

_NC_CACHE = {}
LAST_RESULT = None  # BassKernelResults of the most recent kernel() call


def _build_nc(s: int = S, num_devices: int = N_CORES):
    P = 128
    NQ = s // 512
    NS = s // P
    ND = D // P
    NT = QC // P
    VW = HD + 1  # 65: per-head V block width (64 cols + ones col)
    VPAD = 7 * VW + P  # 583: last head's lhsT slice must fit

    nc = bacc.Bacc("TRN2", target_bir_lowering=False, debug=False, num_devices=num_devices)

    xT_d = nc.dram_tensor("xT", [D, s], BF16, kind="ExternalInput").ap()
    wq_d = nc.dram_tensor("wq", [D, QC], BF16, kind="ExternalInput").ap()
    wk_d = nc.dram_tensor("wk", [D, QC], BF16, kind="ExternalInput").ap()
    wv_d = nc.dram_tensor("wv", [D, QC], BF16, kind="ExternalInput").ap()
    wo_d = nc.dram_tensor("wo", [QC, D], BF16, kind="ExternalInput").ap()
    bq_d = nc.dram_tensor("bq", [QC], F32, kind="ExternalInput").ap()
    ones_d = nc.dram_tensor("ones2d", [P, 512], F32, kind="ExternalInput").ap()
    zeros_d = nc.dram_tensor("zeros2d", [P, 4096], F32, kind="ExternalInput").ap()
    out_d = nc.dram_tensor("out", [s, D], F32, kind="ExternalOutput").ap()

    with tile.TileContext(nc) as tc:
        with ExitStack() as ctx:
            consts = ctx.enter_context(tc.tile_pool(name="consts", bufs=1))
            persist = ctx.enter_context(tc.tile_pool(name="persist", bufs=1))
            e_pool = ctx.enter_context(tc.tile_pool(name="epool", bufs=5))
            n_pool = ctx.enter_context(tc.tile_pool(name="npool", bufs=4))
            b_pool = ctx.enter_context(tc.tile_pool(name="bpool", bufs=4))
            o_pool = ctx.enter_context(tc.tile_pool(name="opool", bufs=3))
            proj_psum = ctx.enter_context(tc.tile_pool(name="proj_ps", bufs=2, space="PSUM"))
            s_psum = ctx.enter_context(tc.tile_pool(name="s_ps", bufs=2, space="PSUM"))
            a_psum = ctx.enter_context(tc.tile_pool(name="a_ps", bufs=2, space="PSUM"))

            # --- warmup path: depends only on the small ones DMA ---
            ones_t = consts.tile([P, 512], F32)
            nc.sync.dma_start(ones_t[:], ones_d)
            ones_b = consts.tile([P, 512], BF16)
            nc.any.tensor_copy(ones_b[:], ones_t[:])

            warm = proj_psum.tile([P, 512], F32, tag="pp", name="warm")
            for _ in range(48):
                nc.tensor.matmul(
                    warm[:], lhsT=ones_b[:, 0:P], rhs=ones_b[:], start=True, stop=True
                )

            # --- input DMAs, ordered by first use ---
            bqc = consts.tile([P, NT], F32)
            nc.sync.dma_start(bqc[:], bq_d.rearrange("(t p) -> p t", p=P))

            QT = persist.tile([P, NT, s], BF16)
            KTz = persist.tile([P, HG, s], BF16)
            V = persist.tile([P, NS, VPAD + 1], BF16)
            AT = persist.tile([P, NT, s], BF16)
            xT = persist.tile([P, ND, s], BF16)
            wq_sb = persist.tile([P, ND, QC], BF16)
            wk_sb = persist.tile([P, ND, QC], BF16)
            wv_sb = persist.tile([P, ND, QC], BF16)
            wo_sb = persist.tile([P, NT, D], BF16)

            for d in range(ND):
                nc.sync.dma_start(wq_sb[:, d, :], wq_d[d * P : (d + 1) * P, :])
                nc.sync.dma_start(xT[:, d, :], xT_d[d * P : (d + 1) * P, :])
            for d in range(ND):
                nc.sync.dma_start(wk_sb[:, d, :], wk_d[d * P : (d + 1) * P, :])
            for d in range(ND):
                nc.sync.dma_start(wv_sb[:, d, :], wv_d[d * P : (d + 1) * P, :])

            # zero the pad rows of KTz; zero V pad cols; set V ones columns
            zee = zeros_d.bitcast(BF16)
            for t in range(NT):
                nc.sync.dma_start(KTz[64:128, 2 * t, :], zee[64:128, 0:s])
                nc.sync.dma_start(KTz[0:64, 2 * t + 1, :], zee[0:64, 0:s])
            nc.sync.dma_start(
                V[:, :, 7 * VW + HD + 1 :],
                zee[:, 0 : NS * (P - HD)].rearrange("p (a b) -> p a b", a=NS),
            )
            nc.any.tensor_copy(
                V[:, :, 0 : HG * VW].rearrange("p s (h c) -> p s h c", c=VW)[:, :, :, HD : HD + 1],
                ones_t[:, 0 : NS * HG].rearrange("p (a b c) -> p a b c", a=NS, b=HG),
            )

            nc.sync.dma_start(wo_sb[:], wo_d.rearrange("(nt p) c -> p nt c", p=P))

            tri = consts.tile([P, P], F32)
            make_upper_triangular(nc, tri[:], val=1.0, diag=True)
            tri_b = consts.tile([P, P], BF16)
            nc.any.tensor_copy(tri_b[:], tri[:])

            def proj_group(j, g):
                """One psum-group of the j-chunk projections; g in 0..11."""
                js = slice(j * 512, (j + 1) * 512)
                kind, t = divmod(g, NT)
                ps = proj_psum.tile([P, 512], F32, tag="pp", name="pp")
                if kind == 0:  # Q
                    for d in range(ND):
                        nc.tensor.matmul(
                            ps[:],
                            lhsT=wq_sb[:, d, t * P : (t + 1) * P],
                            rhs=xT[:, d, js],
                            start=(d == 0),
                            stop=(d == ND - 1),
                        )
                    nc.vector.tensor_scalar_add(QT[:, t, js], ps[:], bqc[:, t : t + 1])
                elif kind == 1:  # K (no bias: softmax shift-invariance)
                    for d in range(ND):
                        nc.tensor.matmul(
                            ps[:],
                            lhsT=wk_sb[:, d, t * P : (t + 1) * P],
                            rhs=xT[:, d, js],
                            start=(d == 0),
                            stop=(d == ND - 1),
                        )
                    nc.vector.tensor_copy(KTz[0:64, 2 * t, js], ps[0:64, :])
                    nc.vector.tensor_copy(KTz[64:128, 2 * t + 1, js], ps[64:128, :])
                else:  # V s-tile 4j+t (no bias: folded into bo_eff host-side)
                    st = 4 * j + t
                    for d in range(ND):
                        nc.tensor.matmul(
                            ps[:],
                            lhsT=xT[:, d, st * P : (st + 1) * P],
                            rhs=wv_sb[:, d, :],
                            start=(d == 0),
                            stop=(d == ND - 1),
                        )
                    dst = V[:, st, 0 : HG * VW].rearrange("p (h c) -> p h c", c=VW)[:, :, 0:HD]
                    src = ps.rearrange("p (h c) -> p h c", c=HD)
                    nc.vector.tensor_copy(dst, src)

            def attn_head(j, h):
                t, half = h // 2, h % 2
                pb = 64 * half
                nkb = 4 * j + 4
                nfull = 4 * j
                js0 = j * 512
                A_ps = a_psum.tile([P, 512], F32, tag="A", name="A")
                # full k-blocks, in pairs sharing one 2-bank psum tile + one exp
                for p0 in range(0, nfull, 2):
                    s2 = s_psum.tile([P, 2, 512], F32, name="s2")
                    for i in (0, 1):
                        kb = p0 + i
                        nc.tensor.matmul(
                            s2[:, i, :],
                            lhsT=KTz[:, h, kb * P : (kb + 1) * P],
                            rhs=QT[:, t, js0 : js0 + 512],
                            start=True,
                            stop=True,
                        )
                    E = e_pool.tile([P, 2, 512], BF16, name="E")
                    nc.scalar.activation(E[:], s2[:], AF.Exp, scale=0.125)
                    for i in (0, 1):
                        kb = p0 + i
                        nc.tensor.matmul(
                            A_ps[:],
                            lhsT=V[:, kb, h * VW : h * VW + P],
                            rhs=E[:, i, :],
                            start=(kb == 0),
                            stop=False,
                        )
                # diagonal k-blocks: pair psum tile, separate trimmed exps
                for p0 in range(nfull, nkb, 2):
                    s2 = s_psum.tile([P, 2, 512], F32, name="s2")
                    for i in (0, 1):
                        kb = p0 + i
                        y0 = P * (kb - 4 * j)
                        nc.tensor.matmul(
                            s2[:, i, y0:],
                            lhsT=KTz[:, h, kb * P : (kb + 1) * P],
                            rhs=QT[:, t, js0 + y0 : js0 + 512],
                            start=True,
                            stop=True,
                        )
                    E = e_pool.tile([P, 2, 512], BF16, name="E")
                    for i in (0, 1):
                        kb = p0 + i
                        y0 = P * (kb - 4 * j)
                        nc.scalar.activation(E[:, i, y0:], s2[:, i, y0:], AF.Exp, scale=0.125)
                        nc.vector.tensor_tensor(
                            E[:, i, y0 : y0 + P], E[:, i, y0 : y0 + P], tri_b[:], ALU.mult
                        )
                        nc.tensor.matmul(
                            A_ps[:, y0:],
                            lhsT=V[:, kb, h * VW : h * VW + P],
                            rhs=E[:, i, y0:],
                            start=(kb == 0),
                            stop=(kb == nkb - 1),
                        )
                # normalize: reciprocal of the denominator row, broadcast
                # (base partition 0 only -- offset dst crashes gpsimd), then
                # one DVE multiply straight out of PSUM into AT
                sums = n_pool.tile([1, 512], F32, tag="sums", name="sums")
                nc.vector.tensor_copy(sums[:], A_ps[HD : HD + 1, :])
                rec = n_pool.tile([1, 512], F32, tag="rec", name="rec")
                nc.vector.reciprocal_approx_fast(rec[:], sums[:])
                bc = b_pool.tile([HD, 512], F32, name="bc")
                nc.gpsimd.partition_broadcast(bc[:], rec[0:1, :])
                nc.vector.tensor_tensor(
                    AT[pb : pb + HD, t, js0 : js0 + 512],
                    A_ps[0:HD, :],
                    bc[:],
                    ALU.mult,
                )

            def out_proj_group(j, g):
                st = 4 * j + g // 2
                oc = g % 2
                o_ps = proj_psum.tile([P, 512], F32, tag="pp", name="o_ps")
                for t2 in range(NT):
                    nc.tensor.matmul(
                        o_ps[:],
                        lhsT=AT[:, t2, st * P : (st + 1) * P],
                        rhs=wo_sb[:, t2, oc * 512 : (oc + 1) * 512],
                        start=(t2 == 0),
                        stop=(t2 == NT - 1),
                    )
                ot = o_pool.tile([P, 512], F32, name="ot")
                nc.vector.tensor_copy(ot[:], o_ps[:])
                nc.sync.dma_start(
                    out_d[st * P : (st + 1) * P, oc * 512 : (oc + 1) * 512], ot[:]
                )

            # j-chunk 0 projections up front; then pipeline: attention(j)
            # interleaved with the projections of chunk j+1 at head granularity
            for g in range(12):
                proj_group(0, g)
            for j in range(NQ):
                # filler between heads: chunk j+1 projections, then chunk j-1
                # output projection (ready since attn(j-1) completed)
                filler = [("p", j + 1, g) for g in range(12)] if j + 1 < NQ else []
                if j > 0:
                    filler += [("o", j - 1, g) for g in range(8)]
                for h in range(HG):
                    attn_head(j, h)
                    k0 = (len(filler) * h) // HG
                    k1 = (len(filler) * (h + 1)) // HG
                    for kind, jj, g in filler[k0:k1]:
                        if kind == "p":
                            proj_group(jj, g)
                        else:
                            out_proj_group(jj, g)
                    if j == NQ - 1 and h < HG - 1:
                        # no filler work in the last chunk: two dummy matmuls
                        # keep the PE HAM clock gate warm across exp waits
                        kw = proj_psum.tile([P, 512], F32, tag="pp", name="kw")
                        for _ in range(2):
                            nc.tensor.matmul(
                                kw[:, 0:P], lhsT=ones_b[:, 0:P], rhs=ones_b[:, 0:P],
                                start=True, stop=True,
                            )
            # tail: chunk 3's output projection, double-buffered in proj_psum
            for g in range(8):
                out_proj_group(NQ - 1, g)

    nc.compile()

    return nc


def _get_nc():
    if "nc" not in _NC_CACHE:
        _NC_CACHE["nc"] = _build_nc()
    return _NC_CACHE["nc"]


def make_in_maps(x, wq, bq, wk, bk, wv, bv, wo, bo, n_cores=N_CORES):
    import ml_dtypes

    bf = ml_dtypes.bfloat16
    x = np.asarray(x, np.float32).astype(bf)
    wq, wk, wv, wo = (np.asarray(a, np.float32).astype(bf) for a in (wq, wk, wv, wo))
    bq = np.asarray(bq, np.float32)
    ones2d = np.ones((128, 512), np.float32)
    zeros2d = np.zeros((128, 4096), np.float32)
    in_maps = []
    for c in range(n_cores):
        b, g = c // 2, c % 2
        cs = slice(g * QC, (g + 1) * QC)
        in_maps.append(
            {
                "xT": np.ascontiguousarray(x[b].T),
                "wq": np.ascontiguousarray(wq[:, cs]),
                "wk": np.ascontiguousarray(wk[:, cs]),
                "wv": np.ascontiguousarray(wv[:, cs]),
                "wo": np.ascontiguousarray(wo[cs, :]),
                "bq": np.ascontiguousarray(bq[cs]),
                "ones2d": ones2d,
                "zeros2d": zeros2d,
            }
        )
    return in_maps


def kernel(x, wq, bq, wk, bk, wv, bv, wo, bo):
    global LAST_RESULT
    in_maps = make_in_maps(x, wq, bq, wk, bk, wv, bv, wo, bo)
    nc = _get_nc()
    trace = os.environ.get("MHA_TRACE", "0") == "1"
    res = run_bass_kernel_spmd(nc, in_maps, core_ids=list(range(N_CORES)), trace=trace)
    LAST_RESULT = res

    # bv and bo are folded in host-side: out += bo + bv @ wo (exact in fp32;
    # the V bias passes through softmax normalization unchanged)
    bo_full = (
        np.asarray(bo, np.float32)
        + np.asarray(bv, np.float32) @ np.asarray(wo, np.float32)
    )
    out = np.empty((B, S, D), np.float32)
    for b in range(B):
        out[b] = res.results[2 * b]["out"] + res.results[2 * b + 1]["out"] + bo_full
    return out
